# revision 4
# baseline (speedup 1.0000x reference)
"""BFFN (linear-attention style gated FFN) Trainium2 Bass kernel, 8 NeuronCores.

Reference computation (all fp32, B=4, N=4096, D=E=1024):
    query = (x_real @ Wqr) * (x_imag @ Wqi)        # [b, n, e]
    key   = x_real @ Wk                             # [b, n, d]
    value = x_imag @ Wv                             # [b, n, e]
    kv    = einsum('bnd,bne->bde', key, value)      # [b, d, e]
    out   = einsum('bnd,bde->bne', query, kv)       # [b, n, e]

Key algebraic restructure: kv = Wk^T @ (xr^T @ xi) @ Wv.  With
S = xr^T @ xi (the only sequence-length reduction), the kv path costs
N*D*D + 2*D*D*E instead of 3*N*D*E FLOPs, and S is computed from x in
NATURAL layout (lhsT = xr tile, rhs = xi tile — no transposes needed).

Sharding: 8 cores = 4 batches x 2 sequence-halves. Each pair AllReduces its
partial S (bf16, 2MB) while the query matmuls run; both cores then compute
kv = Wk^T S Wv redundantly (small) and their own half of the output.

S is [1024,1024] fp32 = 4MB > PSUM (2MB), so it accumulates over the
sequence in two passes of 8 PSUM banks each (d' columns 0:512 then
512:1024).  Pass A streams x f32 from HBM (HWDGE) and casts to bf16 on DVE;
it also writes a bf16 copy of x back to HBM so pass B re-reads half the
bytes with no cast.  The xbar DMA transposes building xT (for the query
path) ride the same pass-A stream.  All on-sync-queue: scalar-engine-issued
xbar transposes were observed to race with their readers.
"""
import numpy as np

import concourse.bass as bass
import concourse.mybir as mybir
import concourse.tile as tile
from concourse import bacc
from concourse.bass import ts, ds
from concourse.bass_utils import run_bass_kernel_spmd

F32 = mybir.dt.float32
BF16 = mybir.dt.bfloat16

B, N, D, E = 4, 4096, 1024, 1024
N_CORES = 8
NL = N // 2          # 2048 rows (sequence) per core
P = 128
NT = NL // P         # 16 n-tiles
DT = D // P          # 8 d tiles
ET = E // P          # 8 e tiles
FD = 512             # matmul moving free dim / PSUM bank
NCH = NL // FD       # 4 n-chunks of 512

REPLICA_GROUPS = [[0, 1], [2, 3], [4, 5], [6, 7]]


def build_bass():
    nc = bacc.Bacc("TRN2", target_bir_lowering=False, debug=False,
                   num_devices=N_CORES)

    xr = nc.dram_tensor("xr", [NL, D], F32, kind="ExternalInput").ap()
    xi = nc.dram_tensor("xi", [NL, D], F32, kind="ExternalInput").ap()
    wqr = nc.dram_tensor("wqr", [D, E], F32, kind="ExternalInput").ap()
    wqi = nc.dram_tensor("wqi", [D, E], F32, kind="ExternalInput").ap()
    wk = nc.dram_tensor("wk", [D, E], F32, kind="ExternalInput").ap()
    wv = nc.dram_tensor("wv", [D, E], F32, kind="ExternalInput").ap()
    out = nc.dram_tensor("out", [NL, E], F32, kind="ExternalOutput").ap()

    def as_tiles(w):  # [1024, n] DRAM view -> [128, 8, n] partition-major
        return w.rearrange("(t p) n -> p t n", p=P)

    with tile.TileContext(nc) as tc:
        with (
            tc.tile_pool(name="xs", bufs=3) as xs_pool,        # f32 staging
            tc.tile_pool(name="xnat", bufs=3) as xnat_pool,    # bf16 x tiles
            tc.tile_pool(name="xnh", bufs=3) as xnh_pool,
            tc.tile_pool(name="xt", bufs=2) as xt_pool,
            tc.tile_pool(name="wp", bufs=2) as w_pool,
            tc.tile_pool(name="qt", bufs=1) as qt_pool,
            tc.tile_pool(name="sm", bufs=2) as sm_pool,
            tc.tile_pool(name="sst", bufs=2) as sst_pool,
            tc.tile_pool(name="prst", bufs=2) as prt_pool,
            tc.tile_pool(name="outst", bufs=2) as out_pool,
            tc.tile_pool(name="ps", bufs=8, space="PSUM") as ps_pool,
            tc.tile_pool(name="dram", bufs=2, space="DRAM") as dram_pool,
        ):
            bounce_in = dram_pool.tile([D, D], BF16, name="bounce_in")
            bounce_out = dram_pool.tile([D, D], BF16, name="bounce_out")
            xr_bf = dram_pool.tile([NL, D], BF16, name="xr_bf")
            xi_bfh = dram_pool.tile([NL, FD], BF16, name="xi_bfh")

            xtr = xt_pool.tile([P, DT, NL], BF16, tag="xt", name="xtr")
            xti = xt_pool.tile([P, DT, NL], BF16, tag="xt", name="xti")

            def load_w(dram_w, name):
                w_sb = w_pool.tile([P, DT, E], BF16, tag="w", name=name)
                for t in range(DT):
                    wst = xs_pool.tile([P, D], F32, tag="xs", name="wst")
                    nc.sync.dma_start(wst[:], dram_w[ts(t, P), :])
                    nc.gpsimd.tensor_copy(w_sb[:, t, :], wst[:])
                return w_sb

            # ---- pass A over n: load+cast x, xbar-transpose, S[:, 0:512],
            #      and write bf16 x cache for pass B ----
            ps_s = [ps_pool.tile([P, FD], F32, tag="ps", name="ps_s")
                    for _ in range(DT)]
            for nt in range(NT):
                xs_r = xs_pool.tile([P, D], F32, tag="xs", name="xs_r")
                nc.sync.dma_start(xs_r[:], xr[ts(nt, P), :])
                xr_nat = xnat_pool.tile([P, D], BF16, tag="xn", name="xr_nat")
                nc.vector.tensor_copy(xr_nat[:], xs_r[:])
                xs_i = xs_pool.tile([P, D], F32, tag="xs", name="xs_i")
                nc.sync.dma_start(xs_i[:], xi[ts(nt, P), :])
                xi_nat = xnat_pool.tile([P, D], BF16, tag="xn", name="xi_nat")
                nc.vector.tensor_copy(xi_nat[:], xs_i[:])

                nc.sync.dma_start(xtr[:, :, ts(nt, P)], xr_nat[:],
                                  transpose=True)
                nc.sync.dma_start(xti[:, :, ts(nt, P)], xi_nat[:],
                                  transpose=True)
                nc.sync.dma_start(xr_bf[ts(nt, P), :], xr_nat[:])
                nc.sync.dma_start(xi_bfh[ts(nt, P), :], xi_nat[:, FD:])
                for d in range(DT):
                    nc.tensor.matmul(
                        ps_s[d][:], xr_nat[:, ts(d, P)], xi_nat[:, :FD],
                        start=(nt == 0), stop=(nt == NT - 1),
                    )
                if nt == 2:
                    wqr_sb = load_w(wqr, "wqr_sb")
                    wqi_sb = load_w(wqi, "wqi_sb")
            for d in range(DT):
                s_st = sst_pool.tile([P, FD], BF16, tag="sst", name="s_st")
                nc.vector.tensor_copy(s_st[:], ps_s[d][:])
                nc.sync.dma_start(bounce_in[ts(d, P), :FD], s_st[:])

            # ---- pass B over n (bf16 cache reads): S[:, 512:1024] ----
            ps_s2 = [ps_pool.tile([P, FD], F32, tag="ps", name="ps_s2")
                     for _ in range(DT)]
            for nt in range(NT):
                xr_nat2 = xnat_pool.tile([P, D], BF16, tag="xn", name="xr_nat2")
                nc.sync.dma_start(xr_nat2[:], xr_bf[ts(nt, P), :])
                xi_h = xnh_pool.tile([P, FD], BF16, tag="xnh", name="xi_h")
                nc.sync.dma_start(xi_h[:], xi_bfh[ts(nt, P), :])
                for d in range(DT):
                    nc.tensor.matmul(
                        ps_s2[d][:], xr_nat2[:, ts(d, P)], xi_h[:],
                        start=(nt == 0), stop=(nt == NT - 1),
                    )
            for d in range(DT):
                s_st2 = sst_pool.tile([P, FD], BF16, tag="sst", name="s_st2")
                nc.vector.tensor_copy(s_st2[:], ps_s2[d][:])
                nc.sync.dma_start(bounce_in[ts(d, P), FD:], s_st2[:])

            # ---- pairwise AllReduce of S (bf16, 2MB) ----
            nc.gpsimd.collective_compute(
                "AllReduce",
                mybir.AluOpType.add,
                replica_groups=REPLICA_GROUPS,
                ins=[bounce_in.opt()],
                outs=[bounce_out.opt()],
            )

            # ---- queryT (overlaps the collective): [e_q, n] ----
            qt_sb = qt_pool.tile([P, ET, NL], BF16, tag="qt", name="qt_sb")
            for et in range(ET):
                for nch in range(NCH):
                    ps_r = ps_pool.tile([P, FD], F32, tag="ps", name="ps_qr")
                    for d in range(DT):
                        nc.tensor.matmul(
                            ps_r[:], wqr_sb[:, d, ts(et, P)],
                            xtr[:, d, ts(nch, FD)],
                            start=(d == 0), stop=(d == DT - 1),
                        )
                    prt = prt_pool.tile([P, FD], BF16, tag="prt", name="prt")
                    nc.vector.tensor_copy(prt[:], ps_r[:])
                    ps_i = ps_pool.tile([P, FD], F32, tag="ps", name="ps_qi")
                    for d in range(DT):
                        nc.tensor.matmul(
                            ps_i[:], wqi_sb[:, d, ts(et, P)],
                            xti[:, d, ts(nch, FD)],
                            start=(d == 0), stop=(d == DT - 1),
                        )
                    nc.vector.tensor_mul(
                        out=qt_sb[:, et, ts(nch, FD)], in0=prt[:], in1=ps_i[:],
                    )
                if et == 0:
                    wk_sb = load_w(wk, "wk_sb")
                    wv_sb = load_w(wv, "wv_sb")

            # ---- kv = Wk^T S Wv from the reduced S ----
            s_sb = sm_pool.tile([P, DT, D], BF16, tag="sm", name="s_sb")
            nc.sync.dma_start(s_sb[:], as_tiles(bounce_out))

            ut_sb = sm_pool.tile([P, DT, D], BF16, tag="sm", name="ut_sb")
            for dpt in range(DT):      # d' tile (UT partition dim)
                ps_u = [ps_pool.tile([P, FD], F32, tag="ps", name="ps_u")
                        for _ in range(2)]
                for d in range(DT):
                    lhsT = s_sb[:, d, ts(dpt, P)]
                    for kh in range(2):
                        nc.tensor.matmul(
                            ps_u[kh][:], lhsT, wk_sb[:, d, ts(kh, FD)],
                            start=(d == 0), stop=(d == DT - 1),
                        )
                for kh in range(2):
                    nc.vector.tensor_copy(ut_sb[:, dpt, ts(kh, FD)],
                                          ps_u[kh][:])

            kv_sb = sm_pool.tile([P, DT, E], BF16, tag="sm", name="kv_sb")
            for dkt in range(DT):      # dk tile (kv partition dim)
                ps_k = [ps_pool.tile([P, FD], F32, tag="ps", name="ps_k")
                        for _ in range(2)]
                for dp in range(DT):
                    lhsT = ut_sb[:, dp, ts(dkt, P)]
                    for eh in range(2):
                        nc.tensor.matmul(
                            ps_k[eh][:], lhsT, wv_sb[:, dp, ts(eh, FD)],
                            start=(dp == 0), stop=(dp == DT - 1),
                        )
                for eh in range(2):
                    nc.vector.tensor_copy(kv_sb[:, dkt, ts(eh, FD)],
                                          ps_k[eh][:])

            # ---- out = queryT.T @ kv ----
            for nt in range(NT):
                ps_o = [ps_pool.tile([P, FD], F32, tag="ps", name="ps_o")
                        for _ in range(2)]
                for et in range(ET):
                    lhsT = qt_sb[:, et, ts(nt, P)]
                    for eh in range(2):
                        nc.tensor.matmul(
                            ps_o[eh][:], lhsT, kv_sb[:, et, ts(eh, FD)],
                            start=(et == 0), stop=(et == ET - 1),
                        )
                for eh in range(2):
                    o_st = out_pool.tile([P, FD], F32, tag="ost", name="o_st")
                    nc.vector.tensor_copy(o_st[:], ps_o[eh][:])
                    nc.sync.dma_start(out[ts(nt, P), ts(eh, FD)], o_st[:])

    nc.compile()
    return nc


def make_in_maps(x_real, x_imag, w_query_real, w_query_imag, w_key, w_value):
    ws = {
        "wqr": np.ascontiguousarray(w_query_real, dtype=np.float32),
        "wqi": np.ascontiguousarray(w_query_imag, dtype=np.float32),
        "wk": np.ascontiguousarray(w_key, dtype=np.float32),
        "wv": np.ascontiguousarray(w_value, dtype=np.float32),
    }
    in_maps = []
    for c in range(N_CORES):
        b, h = divmod(c, 2)
        sl = slice(h * NL, (h + 1) * NL)
        in_maps.append({
            "xr": np.ascontiguousarray(x_real[b, sl], dtype=np.float32),
            "xi": np.ascontiguousarray(x_imag[b, sl], dtype=np.float32),
            **ws,
        })
    return in_maps


def gather_out(results):
    out = np.empty((B, N, E), np.float32)
    for c in range(N_CORES):
        b, h = divmod(c, 2)
        out[b, h * NL:(h + 1) * NL] = results[c]["out"]
    return out


def kernel(x_real, x_imag, w_query_real, w_query_imag, w_key, w_value):
    nc = build_bass()
    in_maps = make_in_maps(x_real, x_imag, w_query_real, w_query_imag,
                           w_key, w_value)
    res = run_bass_kernel_spmd(nc, in_maps, core_ids=list(range(N_CORES)))
    return gather_out(res.results)


if __name__ == "__main__":
    rng = np.random.default_rng(0)
    args = dict(
        x_real=rng.standard_normal((B, N, D), dtype=np.float32),
        x_imag=rng.standard_normal((B, N, D), dtype=np.float32),
        w_query_real=(rng.standard_normal((D, E), dtype=np.float32) / D),
        w_query_imag=(rng.standard_normal((D, E), dtype=np.float32) / D),
        w_key=(rng.standard_normal((D, E), dtype=np.float32) / D),
        w_value=(rng.standard_normal((D, E), dtype=np.float32) / D),
    )
    got = kernel(**args)
    q = np.einsum("bnd,de->bne", args["x_real"], args["w_query_real"]) * \
        np.einsum("bnd,de->bne", args["x_imag"], args["w_query_imag"])
    k = np.einsum("bnd,de->bne", args["x_real"], args["w_key"])
    v = np.einsum("bnd,de->bne", args["x_imag"], args["w_value"])
    kv = np.einsum("bnd,bne->bde", k, v)
    want = np.einsum("bnd,bde->bne", q, kv)
    denom = np.abs(want).max()
    print("max abs err:", np.abs(got - want).max())
    print("rel err:", np.abs(got - want).max() / denom)


# revision 5
# speedup vs baseline: 1.0863x; 1.0863x over previous
"""BFFN (linear-attention style gated FFN) Trainium2 Bass kernel, 8 NeuronCores.

Reference computation (all fp32, B=4, N=4096, D=E=1024):
    query = (x_real @ Wqr) * (x_imag @ Wqi)        # [b, n, e]
    key   = x_real @ Wk                             # [b, n, d]
    value = x_imag @ Wv                             # [b, n, e]
    kv    = einsum('bnd,bne->bde', key, value)      # [b, d, e]
    out   = einsum('bnd,bde->bne', query, kv)       # [b, n, e]

Key algebraic restructure: kv = Wk^T @ (xr^T @ xi) @ Wv.  With
S = xr^T @ xi (the only sequence-length reduction), the kv path costs
N*D*D + 2*D*D*E instead of 3*N*D*E FLOPs, and S is computed from x in
NATURAL layout (lhsT = xr tile, rhs = xi tile — no transposes needed).

Sharding: 8 cores = 4 batches x 2 sequence-halves. Each pair AllReduces its
partial S (bf16, 2MB) while the query matmuls run; both cores then compute
kv = Wk^T S Wv redundantly (small) and their own half of the output.

S is [1024,1024] fp32 = 4MB > PSUM (2MB), so it accumulates over the
sequence in two passes of 8 PSUM banks each (d' columns 0:512 then
512:1024).  Pass A streams x f32 from HBM (HWDGE) and casts to bf16 on DVE;
it also writes a bf16 copy of x back to HBM so pass B re-reads half the
bytes with no cast.  The xbar DMA transposes building xT (for the query
path) ride the same pass-A stream.  All on-sync-queue: scalar-engine-issued
xbar transposes were observed to race with their readers.
"""
import numpy as np

import concourse.bass as bass
import concourse.mybir as mybir
import concourse.tile as tile
from concourse import bacc
from concourse.bass import ts, ds
from concourse.bass_utils import run_bass_kernel_spmd

F32 = mybir.dt.float32
BF16 = mybir.dt.bfloat16

B, N, D, E = 4, 4096, 1024, 1024
N_CORES = 8
NL = N // 2          # 2048 rows (sequence) per core
P = 128
NT = NL // P         # 16 n-tiles
DT = D // P          # 8 d tiles
ET = E // P          # 8 e tiles
FD = 512             # matmul moving free dim / PSUM bank
NCH = NL // FD       # 4 n-chunks of 512

REPLICA_GROUPS = [[0, 1], [2, 3], [4, 5], [6, 7]]


def build_bass():
    nc = bacc.Bacc("TRN2", target_bir_lowering=False, debug=False,
                   num_devices=N_CORES)

    xr = nc.dram_tensor("xr", [NL, D], F32, kind="ExternalInput").ap()
    xi = nc.dram_tensor("xi", [NL, D], F32, kind="ExternalInput").ap()
    wqr = nc.dram_tensor("wqr", [D, E], F32, kind="ExternalInput").ap()
    wqi = nc.dram_tensor("wqi", [D, E], F32, kind="ExternalInput").ap()
    wk = nc.dram_tensor("wk", [D, E], F32, kind="ExternalInput").ap()
    wv = nc.dram_tensor("wv", [D, E], F32, kind="ExternalInput").ap()
    out = nc.dram_tensor("out", [NL, E], F32, kind="ExternalOutput").ap()

    def as_tiles(w):  # [1024, n] DRAM view -> [128, 8, n] partition-major
        return w.rearrange("(t p) n -> p t n", p=P)

    with tile.TileContext(nc) as tc:
        with (
            tc.tile_pool(name="xs", bufs=3) as xs_pool,        # f32 staging
            tc.tile_pool(name="xnat", bufs=4) as xnat_pool,    # bf16 x tiles
            tc.tile_pool(name="xnh", bufs=3) as xnh_pool,
            tc.tile_pool(name="xt", bufs=2) as xt_pool,
            tc.tile_pool(name="wp", bufs=2) as w_pool,
            tc.tile_pool(name="qt", bufs=1) as qt_pool,
            tc.tile_pool(name="sm", bufs=2) as sm_pool,
            tc.tile_pool(name="sst", bufs=2) as sst_pool,
            tc.tile_pool(name="prst", bufs=2) as prt_pool,
            tc.tile_pool(name="outst", bufs=2) as out_pool,
            tc.tile_pool(name="ps", bufs=8, space="PSUM") as ps_pool,
            tc.tile_pool(name="dram", bufs=2, space="DRAM") as dram_pool,
        ):
            bounce_in = dram_pool.tile([D, D], BF16, name="bounce_in")
            bounce_out = dram_pool.tile([D, D], BF16, name="bounce_out")
            xr_bf = dram_pool.tile([NL, D], BF16, name="xr_bf")
            xi_bfh = dram_pool.tile([NL, FD], BF16, name="xi_bfh")

            xtr = xt_pool.tile([P, DT, NL], BF16, tag="xt", name="xtr")
            xti = xt_pool.tile([P, DT, NL], BF16, tag="xt", name="xti")

            def load_w(dram_w, name):
                w_sb = w_pool.tile([P, DT, E], BF16, tag="w", name=name)
                nc.gpsimd.dma_start(w_sb[:], as_tiles(dram_w))
                return w_sb

            # ---- pass A over n: load+cast x, xbar-transpose, S[:, 0:512],
            #      and write bf16 x cache for pass B ----
            ps_s = [ps_pool.tile([P, FD], F32, tag="ps", name="ps_s")
                    for _ in range(DT)]
            for nt in range(NT):
                xs_r = xs_pool.tile([P, D], F32, tag="xs", name="xs_r")
                nc.scalar.dma_start(xs_r[:], xr[ts(nt, P), :])
                xr_nat = xnat_pool.tile([P, D], BF16, tag="xn", name="xr_nat")
                nc.vector.tensor_copy(xr_nat[:], xs_r[:])
                xs_i = xs_pool.tile([P, D], F32, tag="xs", name="xs_i")
                nc.scalar.dma_start(xs_i[:], xi[ts(nt, P), :])
                xi_nat = xnat_pool.tile([P, D], BF16, tag="xn", name="xi_nat")
                nc.vector.tensor_copy(xi_nat[:], xs_i[:])

                nc.sync.dma_start(xtr[:, :, ts(nt, P)], xr_nat[:],
                                  transpose=True)
                nc.sync.dma_start(xti[:, :, ts(nt, P)], xi_nat[:],
                                  transpose=True)
                nc.scalar.dma_start(xr_bf[ts(nt, P), :], xr_nat[:])
                nc.scalar.dma_start(xi_bfh[ts(nt, P), :], xi_nat[:, FD:])
                for d in range(DT):
                    nc.tensor.matmul(
                        ps_s[d][:], xr_nat[:, ts(d, P)], xi_nat[:, :FD],
                        start=(nt == 0), stop=(nt == NT - 1),
                    )
                if nt == 2:
                    wqr_sb = load_w(wqr, "wqr_sb")
                    wqi_sb = load_w(wqi, "wqi_sb")
            for d in range(DT):
                s_st = sst_pool.tile([P, FD], BF16, tag="sst", name="s_st")
                nc.vector.tensor_copy(s_st[:], ps_s[d][:])
                nc.scalar.dma_start(bounce_in[ts(d, P), :FD], s_st[:])

            # ---- pass B over n (bf16 cache reads): S[:, 512:1024] ----
            ps_s2 = [ps_pool.tile([P, FD], F32, tag="ps", name="ps_s2")
                     for _ in range(DT)]
            for nt in range(NT):
                xr_nat2 = xnat_pool.tile([P, D], BF16, tag="xn", name="xr_nat2")
                nc.scalar.dma_start(xr_nat2[:], xr_bf[ts(nt, P), :])
                xi_h = xnh_pool.tile([P, FD], BF16, tag="xnh", name="xi_h")
                nc.scalar.dma_start(xi_h[:], xi_bfh[ts(nt, P), :])
                for d in range(DT):
                    nc.tensor.matmul(
                        ps_s2[d][:], xr_nat2[:, ts(d, P)], xi_h[:],
                        start=(nt == 0), stop=(nt == NT - 1),
                    )
            for d in range(DT):
                s_st2 = sst_pool.tile([P, FD], BF16, tag="sst", name="s_st2")
                nc.vector.tensor_copy(s_st2[:], ps_s2[d][:])
                nc.scalar.dma_start(bounce_in[ts(d, P), FD:], s_st2[:])

            # ---- pairwise AllReduce of S (bf16, 2MB) ----
            nc.gpsimd.collective_compute(
                "AllReduce",
                mybir.AluOpType.add,
                replica_groups=REPLICA_GROUPS,
                ins=[bounce_in.opt()],
                outs=[bounce_out.opt()],
            )

            # ---- queryT (overlaps the collective): [e_q, n] ----
            qt_sb = qt_pool.tile([P, ET, NL], BF16, tag="qt", name="qt_sb")
            for et in range(ET):
                for nch in range(NCH):
                    ps_r = ps_pool.tile([P, FD], F32, tag="ps", name="ps_qr")
                    for d in range(DT):
                        nc.tensor.matmul(
                            ps_r[:], wqr_sb[:, d, ts(et, P)],
                            xtr[:, d, ts(nch, FD)],
                            start=(d == 0), stop=(d == DT - 1),
                        )
                    prt = prt_pool.tile([P, FD], BF16, tag="prt", name="prt")
                    nc.vector.tensor_copy(prt[:], ps_r[:])
                    ps_i = ps_pool.tile([P, FD], F32, tag="ps", name="ps_qi")
                    for d in range(DT):
                        nc.tensor.matmul(
                            ps_i[:], wqi_sb[:, d, ts(et, P)],
                            xti[:, d, ts(nch, FD)],
                            start=(d == 0), stop=(d == DT - 1),
                        )
                    nc.vector.tensor_mul(
                        out=qt_sb[:, et, ts(nch, FD)], in0=prt[:], in1=ps_i[:],
                    )
                if et == 0:
                    wk_sb = load_w(wk, "wk_sb")
                    wv_sb = load_w(wv, "wv_sb")

            # ---- kv = Wk^T S Wv from the reduced S ----
            s_sb = sm_pool.tile([P, DT, D], BF16, tag="sm", name="s_sb")
            nc.scalar.dma_start(s_sb[:], as_tiles(bounce_out))

            ut_sb = sm_pool.tile([P, DT, D], BF16, tag="sm", name="ut_sb")
            for dpt in range(DT):      # d' tile (UT partition dim)
                ps_u = [ps_pool.tile([P, FD], F32, tag="ps", name="ps_u")
                        for _ in range(2)]
                for d in range(DT):
                    lhsT = s_sb[:, d, ts(dpt, P)]
                    for kh in range(2):
                        nc.tensor.matmul(
                            ps_u[kh][:], lhsT, wk_sb[:, d, ts(kh, FD)],
                            start=(d == 0), stop=(d == DT - 1),
                        )
                for kh in range(2):
                    nc.vector.tensor_copy(ut_sb[:, dpt, ts(kh, FD)],
                                          ps_u[kh][:])

            kv_sb = sm_pool.tile([P, DT, E], BF16, tag="sm", name="kv_sb")
            for dkt in range(DT):      # dk tile (kv partition dim)
                ps_k = [ps_pool.tile([P, FD], F32, tag="ps", name="ps_k")
                        for _ in range(2)]
                for dp in range(DT):
                    lhsT = ut_sb[:, dp, ts(dkt, P)]
                    for eh in range(2):
                        nc.tensor.matmul(
                            ps_k[eh][:], lhsT, wv_sb[:, dp, ts(eh, FD)],
                            start=(dp == 0), stop=(dp == DT - 1),
                        )
                for eh in range(2):
                    nc.vector.tensor_copy(kv_sb[:, dkt, ts(eh, FD)],
                                          ps_k[eh][:])

            # ---- out = queryT.T @ kv ----
            for nt in range(NT):
                ps_o = [ps_pool.tile([P, FD], F32, tag="ps", name="ps_o")
                        for _ in range(2)]
                for et in range(ET):
                    lhsT = qt_sb[:, et, ts(nt, P)]
                    for eh in range(2):
                        nc.tensor.matmul(
                            ps_o[eh][:], lhsT, kv_sb[:, et, ts(eh, FD)],
                            start=(et == 0), stop=(et == ET - 1),
                        )
                for eh in range(2):
                    o_st = out_pool.tile([P, FD], F32, tag="ost", name="o_st")
                    nc.vector.tensor_copy(o_st[:], ps_o[eh][:])
                    nc.scalar.dma_start(out[ts(nt, P), ts(eh, FD)], o_st[:])

    nc.compile()
    return nc


def make_in_maps(x_real, x_imag, w_query_real, w_query_imag, w_key, w_value):
    ws = {
        "wqr": np.ascontiguousarray(w_query_real, dtype=np.float32),
        "wqi": np.ascontiguousarray(w_query_imag, dtype=np.float32),
        "wk": np.ascontiguousarray(w_key, dtype=np.float32),
        "wv": np.ascontiguousarray(w_value, dtype=np.float32),
    }
    in_maps = []
    for c in range(N_CORES):
        b, h = divmod(c, 2)
        sl = slice(h * NL, (h + 1) * NL)
        in_maps.append({
            "xr": np.ascontiguousarray(x_real[b, sl], dtype=np.float32),
            "xi": np.ascontiguousarray(x_imag[b, sl], dtype=np.float32),
            **ws,
        })
    return in_maps


def gather_out(results):
    out = np.empty((B, N, E), np.float32)
    for c in range(N_CORES):
        b, h = divmod(c, 2)
        out[b, h * NL:(h + 1) * NL] = results[c]["out"]
    return out


def kernel(x_real, x_imag, w_query_real, w_query_imag, w_key, w_value):
    nc = build_bass()
    in_maps = make_in_maps(x_real, x_imag, w_query_real, w_query_imag,
                           w_key, w_value)
    res = run_bass_kernel_spmd(nc, in_maps, core_ids=list(range(N_CORES)))
    return gather_out(res.results)


if __name__ == "__main__":
    rng = np.random.default_rng(0)
    args = dict(
        x_real=rng.standard_normal((B, N, D), dtype=np.float32),
        x_imag=rng.standard_normal((B, N, D), dtype=np.float32),
        w_query_real=(rng.standard_normal((D, E), dtype=np.float32) / D),
        w_query_imag=(rng.standard_normal((D, E), dtype=np.float32) / D),
        w_key=(rng.standard_normal((D, E), dtype=np.float32) / D),
        w_value=(rng.standard_normal((D, E), dtype=np.float32) / D),
    )
    got = kernel(**args)
    q = np.einsum("bnd,de->bne", args["x_real"], args["w_query_real"]) * \
        np.einsum("bnd,de->bne", args["x_imag"], args["w_query_imag"])
    k = np.einsum("bnd,de->bne", args["x_real"], args["w_key"])
    v = np.einsum("bnd,de->bne", args["x_imag"], args["w_value"])
    kv = np.einsum("bnd,bne->bde", k, v)
    want = np.einsum("bnd,bde->bne", q, kv)
    denom = np.abs(want).max()
    print("max abs err:", np.abs(got - want).max())
    print("rel err:", np.abs(got - want).max() / denom)


# revision 6
# speedup vs baseline: 1.0878x; 1.0014x over previous
"""BFFN (linear-attention style gated FFN) Trainium2 Bass kernel, 8 NeuronCores.

Reference computation (all fp32, B=4, N=4096, D=E=1024):
    query = (x_real @ Wqr) * (x_imag @ Wqi)        # [b, n, e]
    key   = x_real @ Wk                             # [b, n, d]
    value = x_imag @ Wv                             # [b, n, e]
    kv    = einsum('bnd,bne->bde', key, value)      # [b, d, e]
    out   = einsum('bnd,bde->bne', query, kv)       # [b, n, e]

Key algebraic restructure: kv = Wk^T @ (xr^T @ xi) @ Wv.  With
S = xr^T @ xi (the only sequence-length reduction), the kv path costs
N*D*D + 2*D*D*E instead of 3*N*D*E FLOPs, and S is computed from x in
NATURAL layout (lhsT = xr tile, rhs = xi tile — no transposes needed).

Sharding: 8 cores = 4 batches x 2 sequence-halves. Each pair AllReduces its
partial S (bf16, 2MB) while the query matmuls run; both cores then compute
kv = Wk^T S Wv redundantly (small) and their own half of the output.

S is [1024,1024] fp32 = 4MB > PSUM (2MB), so it accumulates over the
sequence in two passes of 8 PSUM banks each (d' columns 0:512 then
512:1024).  Pass A streams x f32 from HBM (HWDGE) and casts to bf16 on DVE;
it also writes a bf16 copy of x back to HBM so pass B re-reads half the
bytes with no cast.  The xbar DMA transposes building xT (for the query
path) ride the same pass-A stream.  All on-sync-queue: scalar-engine-issued
xbar transposes were observed to race with their readers.
"""
import numpy as np

import concourse.bass as bass
import concourse.mybir as mybir
import concourse.tile as tile
from concourse import bacc
from concourse.bass import ts, ds
from concourse.bass_utils import run_bass_kernel_spmd

F32 = mybir.dt.float32
BF16 = mybir.dt.bfloat16

B, N, D, E = 4, 4096, 1024, 1024
N_CORES = 8
NL = N // 2          # 2048 rows (sequence) per core
P = 128
NT = NL // P         # 16 n-tiles
DT = D // P          # 8 d tiles
ET = E // P          # 8 e tiles
FD = 512             # matmul moving free dim / PSUM bank
NCH = NL // FD       # 4 n-chunks of 512

REPLICA_GROUPS = [[0, 1], [2, 3], [4, 5], [6, 7]]


def build_bass():
    nc = bacc.Bacc("TRN2", target_bir_lowering=False, debug=False,
                   num_devices=N_CORES)

    xr = nc.dram_tensor("xr", [NL, D], F32, kind="ExternalInput").ap()
    xi = nc.dram_tensor("xi", [NL, D], F32, kind="ExternalInput").ap()
    wqr = nc.dram_tensor("wqr", [D, E], F32, kind="ExternalInput").ap()
    wqi = nc.dram_tensor("wqi", [D, E], F32, kind="ExternalInput").ap()
    wk = nc.dram_tensor("wk", [D, E], F32, kind="ExternalInput").ap()
    wv = nc.dram_tensor("wv", [D, E], F32, kind="ExternalInput").ap()
    out = nc.dram_tensor("out", [NL, E], F32, kind="ExternalOutput").ap()

    def as_tiles(w):  # [1024, n] DRAM view -> [128, 8, n] partition-major
        return w.rearrange("(t p) n -> p t n", p=P)

    with tile.TileContext(nc) as tc:
        with (
            tc.tile_pool(name="xs", bufs=4) as xs_pool,        # f32 staging
            tc.tile_pool(name="xnat", bufs=4) as xnat_pool,
            tc.tile_pool(name="xnh", bufs=3) as xnh_pool,
            tc.tile_pool(name="xt", bufs=2) as xt_pool,
            tc.tile_pool(name="wp", bufs=2) as w_pool,
            tc.tile_pool(name="qt", bufs=1) as qt_pool,
            tc.tile_pool(name="sm", bufs=2) as sm_pool,
            tc.tile_pool(name="sst", bufs=2) as sst_pool,
            tc.tile_pool(name="prst", bufs=2) as prt_pool,
            tc.tile_pool(name="outst", bufs=2) as out_pool,
            tc.tile_pool(name="ps", bufs=8, space="PSUM") as ps_pool,
            tc.tile_pool(name="dram", bufs=2, space="DRAM") as dram_pool,
        ):
            bounce_in = dram_pool.tile([D, D], BF16, name="bounce_in")
            bounce_out = dram_pool.tile([D, D], BF16, name="bounce_out")

            xtr = xt_pool.tile([P, DT, NL], BF16, tag="xt", name="xtr")
            xti = xt_pool.tile([P, DT, NL], BF16, tag="xt", name="xti")

            def load_w(dram_w, name):
                w_sb = w_pool.tile([P, DT, E], BF16, tag="w", name=name)
                nc.gpsimd.dma_start(w_sb[:], as_tiles(dram_w))
                return w_sb

            # ---- pass A over n: load+cast x, xbar-transpose, S[:, 0:512],
            #      and write bf16 x cache for pass B ----
            ps_s = [ps_pool.tile([P, FD], F32, tag="ps", name="ps_s")
                    for _ in range(DT)]
            for nt in range(NT):
                xs_r = xs_pool.tile([P, D], F32, tag="xs", name="xs_r")
                nc.scalar.dma_start(xs_r[:], xr[ts(nt, P), :])
                xr_nat = xnat_pool.tile([P, D], BF16, tag="xn", name="xr_nat")
                nc.vector.tensor_copy(xr_nat[:], xs_r[:])
                xs_i = xs_pool.tile([P, D], F32, tag="xs", name="xs_i")
                nc.scalar.dma_start(xs_i[:], xi[ts(nt, P), :])
                xi_nat = xnat_pool.tile([P, D], BF16, tag="xn", name="xi_nat")
                nc.vector.tensor_copy(xi_nat[:], xs_i[:])

                nc.sync.dma_start(xtr[:, :, ts(nt, P)], xr_nat[:],
                                  transpose=True)
                nc.sync.dma_start(xti[:, :, ts(nt, P)], xi_nat[:],
                                  transpose=True)
                for d in range(DT):
                    nc.tensor.matmul(
                        ps_s[d][:], xr_nat[:, ts(d, P)], xi_nat[:, :FD],
                        start=(nt == 0), stop=(nt == NT - 1),
                    )
                if nt == 2:
                    wqr_sb = load_w(wqr, "wqr_sb")
                    wqi_sb = load_w(wqi, "wqi_sb")
            for d in range(DT):
                s_st = sst_pool.tile([P, FD], BF16, tag="sst", name="s_st")
                nc.vector.tensor_copy(s_st[:], ps_s[d][:])
                nc.scalar.dma_start(bounce_in[ts(d, P), :FD], s_st[:])

            # ---- pass B over n (bf16 cache reads): S[:, 512:1024] ----
            ps_s2 = [ps_pool.tile([P, FD], F32, tag="ps", name="ps_s2")
                     for _ in range(DT)]
            for nt in range(NT):
                xs_r2 = xs_pool.tile([P, D], F32, tag="xs", name="xs_r2")
                nc.scalar.dma_start(xs_r2[:], xr[ts(nt, P), :])
                xr_nat2 = xnat_pool.tile([P, D], BF16, tag="xn", name="xr_nat2")
                nc.vector.tensor_copy(xr_nat2[:], xs_r2[:])
                xs_i2 = xs_pool.tile([P, FD], F32, tag="xs", name="xs_i2")
                nc.scalar.dma_start(xs_i2[:], xi[ts(nt, P), FD:])
                xi_h = xnh_pool.tile([P, FD], BF16, tag="xnh", name="xi_h")
                nc.vector.tensor_copy(xi_h[:], xs_i2[:])
                for d in range(DT):
                    nc.tensor.matmul(
                        ps_s2[d][:], xr_nat2[:, ts(d, P)], xi_h[:],
                        start=(nt == 0), stop=(nt == NT - 1),
                    )
            for d in range(DT):
                s_st2 = sst_pool.tile([P, FD], BF16, tag="sst", name="s_st2")
                nc.vector.tensor_copy(s_st2[:], ps_s2[d][:])
                nc.scalar.dma_start(bounce_in[ts(d, P), FD:], s_st2[:])

            # ---- pairwise AllReduce of S (bf16, 2MB) ----
            nc.gpsimd.collective_compute(
                "AllReduce",
                mybir.AluOpType.add,
                replica_groups=REPLICA_GROUPS,
                ins=[bounce_in.opt()],
                outs=[bounce_out.opt()],
            )

            # ---- queryT (overlaps the collective): [e_q, n] ----
            qt_sb = qt_pool.tile([P, ET, NL], BF16, tag="qt", name="qt_sb")
            for et in range(ET):
                for nch in range(NCH):
                    ps_r = ps_pool.tile([P, FD], F32, tag="ps", name="ps_qr")
                    for d in range(DT):
                        nc.tensor.matmul(
                            ps_r[:], wqr_sb[:, d, ts(et, P)],
                            xtr[:, d, ts(nch, FD)],
                            start=(d == 0), stop=(d == DT - 1),
                        )
                    prt = prt_pool.tile([P, FD], BF16, tag="prt", name="prt")
                    nc.vector.tensor_copy(prt[:], ps_r[:])
                    ps_i = ps_pool.tile([P, FD], F32, tag="ps", name="ps_qi")
                    for d in range(DT):
                        nc.tensor.matmul(
                            ps_i[:], wqi_sb[:, d, ts(et, P)],
                            xti[:, d, ts(nch, FD)],
                            start=(d == 0), stop=(d == DT - 1),
                        )
                    nc.vector.tensor_mul(
                        out=qt_sb[:, et, ts(nch, FD)], in0=prt[:], in1=ps_i[:],
                    )
                if et == 0:
                    wk_sb = load_w(wk, "wk_sb")
                    wv_sb = load_w(wv, "wv_sb")

            # ---- kv = Wk^T S Wv from the reduced S ----
            s_sb = sm_pool.tile([P, DT, D], BF16, tag="sm", name="s_sb")
            nc.scalar.dma_start(s_sb[:], as_tiles(bounce_out))

            ut_sb = sm_pool.tile([P, DT, D], BF16, tag="sm", name="ut_sb")
            for dpt in range(DT):      # d' tile (UT partition dim)
                ps_u = [ps_pool.tile([P, FD], F32, tag="ps", name="ps_u")
                        for _ in range(2)]
                for d in range(DT):
                    lhsT = s_sb[:, d, ts(dpt, P)]
                    for kh in range(2):
                        nc.tensor.matmul(
                            ps_u[kh][:], lhsT, wk_sb[:, d, ts(kh, FD)],
                            start=(d == 0), stop=(d == DT - 1),
                        )
                for kh in range(2):
                    nc.vector.tensor_copy(ut_sb[:, dpt, ts(kh, FD)],
                                          ps_u[kh][:])

            kv_sb = sm_pool.tile([P, DT, E], BF16, tag="sm", name="kv_sb")
            for dkt in range(DT):      # dk tile (kv partition dim)
                ps_k = [ps_pool.tile([P, FD], F32, tag="ps", name="ps_k")
                        for _ in range(2)]
                for dp in range(DT):
                    lhsT = ut_sb[:, dp, ts(dkt, P)]
                    for eh in range(2):
                        nc.tensor.matmul(
                            ps_k[eh][:], lhsT, wv_sb[:, dp, ts(eh, FD)],
                            start=(dp == 0), stop=(dp == DT - 1),
                        )
                for eh in range(2):
                    nc.vector.tensor_copy(kv_sb[:, dkt, ts(eh, FD)],
                                          ps_k[eh][:])

            # ---- out = queryT.T @ kv ----
            for nt in range(NT):
                ps_o = [ps_pool.tile([P, FD], F32, tag="ps", name="ps_o")
                        for _ in range(2)]
                for et in range(ET):
                    lhsT = qt_sb[:, et, ts(nt, P)]
                    for eh in range(2):
                        nc.tensor.matmul(
                            ps_o[eh][:], lhsT, kv_sb[:, et, ts(eh, FD)],
                            start=(et == 0), stop=(et == ET - 1),
                        )
                for eh in range(2):
                    o_st = out_pool.tile([P, FD], F32, tag="ost", name="o_st")
                    nc.vector.tensor_copy(o_st[:], ps_o[eh][:])
                    nc.sync.dma_start(out[ts(nt, P), ts(eh, FD)], o_st[:])

    nc.compile()
    return nc


def make_in_maps(x_real, x_imag, w_query_real, w_query_imag, w_key, w_value):
    ws = {
        "wqr": np.ascontiguousarray(w_query_real, dtype=np.float32),
        "wqi": np.ascontiguousarray(w_query_imag, dtype=np.float32),
        "wk": np.ascontiguousarray(w_key, dtype=np.float32),
        "wv": np.ascontiguousarray(w_value, dtype=np.float32),
    }
    in_maps = []
    for c in range(N_CORES):
        b, h = divmod(c, 2)
        sl = slice(h * NL, (h + 1) * NL)
        in_maps.append({
            "xr": np.ascontiguousarray(x_real[b, sl], dtype=np.float32),
            "xi": np.ascontiguousarray(x_imag[b, sl], dtype=np.float32),
            **ws,
        })
    return in_maps


def gather_out(results):
    out = np.empty((B, N, E), np.float32)
    for c in range(N_CORES):
        b, h = divmod(c, 2)
        out[b, h * NL:(h + 1) * NL] = results[c]["out"]
    return out


def kernel(x_real, x_imag, w_query_real, w_query_imag, w_key, w_value):
    nc = build_bass()
    in_maps = make_in_maps(x_real, x_imag, w_query_real, w_query_imag,
                           w_key, w_value)
    res = run_bass_kernel_spmd(nc, in_maps, core_ids=list(range(N_CORES)))
    return gather_out(res.results)


if __name__ == "__main__":
    rng = np.random.default_rng(0)
    args = dict(
        x_real=rng.standard_normal((B, N, D), dtype=np.float32),
        x_imag=rng.standard_normal((B, N, D), dtype=np.float32),
        w_query_real=(rng.standard_normal((D, E), dtype=np.float32) / D),
        w_query_imag=(rng.standard_normal((D, E), dtype=np.float32) / D),
        w_key=(rng.standard_normal((D, E), dtype=np.float32) / D),
        w_value=(rng.standard_normal((D, E), dtype=np.float32) / D),
    )
    got = kernel(**args)
    q = np.einsum("bnd,de->bne", args["x_real"], args["w_query_real"]) * \
        np.einsum("bnd,de->bne", args["x_imag"], args["w_query_imag"])
    k = np.einsum("bnd,de->bne", args["x_real"], args["w_key"])
    v = np.einsum("bnd,de->bne", args["x_imag"], args["w_value"])
    kv = np.einsum("bnd,bne->bde", k, v)
    want = np.einsum("bnd,bde->bne", q, kv)
    denom = np.abs(want).max()
    print("max abs err:", np.abs(got - want).max())
    print("rel err:", np.abs(got - want).max() / denom)


# revision 8
# speedup vs baseline: 1.3025x; 1.1974x over previous
"""BFFN (linear-attention style gated FFN) Trainium2 Bass kernel, 8 NeuronCores.

Reference computation (all fp32, B=4, N=4096, D=E=1024):
    query = (x_real @ Wqr) * (x_imag @ Wqi)        # [b, n, e]
    key   = x_real @ Wk                             # [b, n, d]
    value = x_imag @ Wv                             # [b, n, e]
    kv    = einsum('bnd,bne->bde', key, value)      # [b, d, e]
    out   = einsum('bnd,bde->bne', query, kv)       # [b, n, e]

Key algebraic restructure: kv = Wk^T @ (xr^T @ xi) @ Wv.  With
S = xr^T @ xi (the only sequence-length reduction), the kv path costs
N*D*D + 2*D*D*E instead of 3*N*D*E FLOPs, and S is computed from x in
NATURAL layout (lhsT = xr tile, rhs = xi tile — no transposes needed).

Sharding: 8 cores = 4 batches x 2 sequence-halves. Each pair AllReduces its
partial S (bf16, 2MB) while the query matmuls run; both cores then compute
kv = Wk^T S Wv redundantly (small) and their own half of the output.

S is [1024,1024] fp32 = 4MB > PSUM (2MB), so it accumulates over the
sequence in two passes of 8 PSUM banks each (d' columns 0:512 then
512:1024).  Pass A streams x f32 (HWDGE) with DVE casts to bf16 and writes
a bf16 copy of x back to HBM; pass B re-reads it with no cast.

The query path needs xT [d, n].  xbar DMA transposes turned out to
serialize against ALL concurrent DMA traffic (Tile's deadlock-avoidance
for the xbar), so xT is built on the TENSOR engine (transpose-via-identity)
chunk by chunk inside the query phase, where PSUM banks are free and the
only DMA is the plain bf16 cache re-read.  xT is never fully resident:
each 512-column chunk is transposed, consumed by the query matmuls, and
dropped.
"""
import numpy as np

import concourse.bass as bass
import concourse.mybir as mybir
import concourse.tile as tile
from concourse import bacc
from concourse.bass import ts, ds
from concourse.bass_utils import run_bass_kernel_spmd
from concourse.masks import make_identity

F32 = mybir.dt.float32
BF16 = mybir.dt.bfloat16

B, N, D, E = 4, 4096, 1024, 1024
N_CORES = 8
NL = N // 2          # 2048 rows (sequence) per core
P = 128
NT = NL // P         # 16 n-tiles
DT = D // P          # 8 d tiles
ET = E // P          # 8 e tiles
FD = 512             # matmul moving free dim / PSUM bank
NCH = NL // FD       # 4 n-chunks of 512

REPLICA_GROUPS = [[0, 1], [2, 3], [4, 5], [6, 7]]


def build_bass():
    nc = bacc.Bacc("TRN2", target_bir_lowering=False, debug=False,
                   num_devices=N_CORES)

    xr = nc.dram_tensor("xr", [NL, D], F32, kind="ExternalInput").ap()
    xi = nc.dram_tensor("xi", [NL, D], F32, kind="ExternalInput").ap()
    wqr = nc.dram_tensor("wqr", [D, E], F32, kind="ExternalInput").ap()
    wqi = nc.dram_tensor("wqi", [D, E], F32, kind="ExternalInput").ap()
    wk = nc.dram_tensor("wk", [D, E], F32, kind="ExternalInput").ap()
    wv = nc.dram_tensor("wv", [D, E], F32, kind="ExternalInput").ap()
    out = nc.dram_tensor("out", [NL, E], F32, kind="ExternalOutput").ap()

    def as_tiles(w):  # [1024, n] DRAM view -> [128, 8, n] partition-major
        return w.rearrange("(t p) n -> p t n", p=P)

    with tile.TileContext(nc) as tc:
        with (
            tc.tile_pool(name="xs", bufs=3) as xs_pool,        # f32 staging
            tc.tile_pool(name="xnat", bufs=8) as xnat_pool,    # bf16 x tiles
            tc.tile_pool(name="xnh", bufs=3) as xnh_pool,
            tc.tile_pool(name="xtc", bufs=4) as xtc_pool,      # xT chunks
            tc.tile_pool(name="wp", bufs=4) as w_pool,
            tc.tile_pool(name="qt", bufs=1) as qt_pool,
            tc.tile_pool(name="sm", bufs=2) as sm_pool,
            tc.tile_pool(name="sst", bufs=2) as sst_pool,
            tc.tile_pool(name="prst", bufs=2) as prt_pool,
            tc.tile_pool(name="outst", bufs=2) as out_pool,
            tc.tile_pool(name="cst", bufs=1) as cst_pool,
            tc.tile_pool(name="ps", bufs=8, space="PSUM") as ps_pool,
            tc.tile_pool(name="dram", bufs=2, space="DRAM") as dram_pool,
        ):
            bounce_in = dram_pool.tile([D, D], BF16, name="bounce_in")
            bounce_out = dram_pool.tile([D, D], BF16, name="bounce_out")
            xr_bf = dram_pool.tile([NL, D], BF16, name="xr_bf")
            xi_bf = dram_pool.tile([NL, D], BF16, name="xi_bf")

            ident = cst_pool.tile([P, P], BF16, tag="id", name="ident")
            make_identity(nc, ident)

            def load_w(dram_w, name):  # SWDGE cast-DMA, f32 -> bf16 in flight
                w_sb = w_pool.tile([P, DT, E], BF16, tag="w", name=name)
                nc.gpsimd.dma_start(w_sb[:], as_tiles(dram_w))
                return w_sb

            # ---- pass A over n: load+cast x, S[:, 0:512], write bf16 cache
            ps_s = [ps_pool.tile([P, FD], F32, tag="ps", name="ps_s")
                    for _ in range(DT)]
            w_sbs = {}
            for nt in range(NT):
                xs_r = xs_pool.tile([P, D], F32, tag="xs", name="xs_r")
                nc.scalar.dma_start(xs_r[:], xr[ts(nt, P), :])
                xr_nat = xnat_pool.tile([P, D], BF16, tag="xn", name="xr_nat")
                nc.vector.tensor_copy(xr_nat[:], xs_r[:])
                xs_i = xs_pool.tile([P, D], F32, tag="xs", name="xs_i")
                nc.scalar.dma_start(xs_i[:], xi[ts(nt, P), :])
                xi_nat = xnat_pool.tile([P, D], BF16, tag="xn", name="xi_nat")
                nc.vector.tensor_copy(xi_nat[:], xs_i[:])

                nc.sync.dma_start(xr_bf[ts(nt, P), :], xr_nat[:])
                nc.sync.dma_start(xi_bf[ts(nt, P), :], xi_nat[:])
                for d in range(DT):
                    nc.tensor.matmul(
                        ps_s[d][:], xr_nat[:, ts(d, P)], xi_nat[:, :FD],
                        start=(nt == 0), stop=(nt == NT - 1),
                    )
                if nt % 4 == 1:  # spread the four weight loads across pass A
                    w_name = ("wqr", "wqi", "wk", "wv")[nt // 4]
                    w_sbs[w_name] = load_w({"wqr": wqr, "wqi": wqi,
                                            "wk": wk, "wv": wv}[w_name],
                                           w_name + "_sb")
            for d in range(DT):
                s_st = sst_pool.tile([P, FD], BF16, tag="sst", name="s_st")
                nc.vector.tensor_copy(s_st[:], ps_s[d][:])
                nc.scalar.dma_start(bounce_in[ts(d, P), :FD], s_st[:])

            # ---- pass B over n (bf16 cache reads): S[:, 512:1024] ----
            ps_s2 = [ps_pool.tile([P, FD], F32, tag="ps", name="ps_s2")
                     for _ in range(DT)]
            for nt in range(NT):
                xr_nat2 = xnat_pool.tile([P, D], BF16, tag="xn", name="xr_nat2")
                nc.scalar.dma_start(xr_nat2[:], xr_bf[ts(nt, P), :])
                xi_h = xnh_pool.tile([P, FD], BF16, tag="xnh", name="xi_h")
                nc.scalar.dma_start(xi_h[:], xi_bf[ts(nt, P), FD:])
                for d in range(DT):
                    nc.tensor.matmul(
                        ps_s2[d][:], xr_nat2[:, ts(d, P)], xi_h[:],
                        start=(nt == 0), stop=(nt == NT - 1),
                    )
            for d in range(DT):
                s_st2 = sst_pool.tile([P, FD], BF16, tag="sst", name="s_st2")
                nc.vector.tensor_copy(s_st2[:], ps_s2[d][:])
                nc.scalar.dma_start(bounce_in[ts(d, P), FD:], s_st2[:])

            # ---- pairwise AllReduce of S (bf16, 2MB); overlaps query ----
            nc.gpsimd.collective_compute(
                "AllReduce",
                mybir.AluOpType.add,
                replica_groups=REPLICA_GROUPS,
                ins=[bounce_in.opt()],
                outs=[bounce_out.opt()],
            )

            # ---- query phase, chunked: PE-transpose xT chunk, then MMs ----
            wqr_sb, wqi_sb = w_sbs["wqr"], w_sbs["wqi"]
            wk_sb, wv_sb = w_sbs["wk"], w_sbs["wv"]
            qt_sb = qt_pool.tile([P, ET, NL], BF16, tag="qt", name="qt_sb")
            for nch in range(NCH):
                chunk_xt = {}
                for src_bf, kind in ((xr_bf, "r"), (xi_bf, "i")):
                    nats = []
                    for j in range(4):  # re-read 4 bf16 x tiles of this chunk
                        xn = xnat_pool.tile([P, D], BF16, tag="xn",
                                            name="xn_c")
                        nc.scalar.dma_start(
                            xn[:], src_bf[ts(4 * nch + j, P), :])
                        nats.append(xn)
                    xt_c = xtc_pool.tile([P, DT, FD], BF16, tag="xtc",
                                         name="xt_c")
                    for d in range(DT):
                        ps_t = ps_pool.tile([P, FD], BF16, tag="ps",
                                            name="ps_t")
                        for j in range(4):
                            nc.tensor.transpose(
                                ps_t[:, ts(j, P)], nats[j][:, ts(d, P)],
                                ident[:],
                            )
                        nc.vector.tensor_copy(xt_c[:, d, :], ps_t[:])
                    chunk_xt[kind] = xt_c
                xtr_c, xti_c = chunk_xt["r"], chunk_xt["i"]
                for et in range(ET):
                    ps_r = ps_pool.tile([P, FD], F32, tag="ps", name="ps_qr")
                    for d in range(DT):
                        nc.tensor.matmul(
                            ps_r[:], wqr_sb[:, d, ts(et, P)], xtr_c[:, d, :],
                            start=(d == 0), stop=(d == DT - 1),
                        )
                    prt = prt_pool.tile([P, FD], BF16, tag="prt", name="prt")
                    nc.vector.tensor_copy(prt[:], ps_r[:])
                    ps_i = ps_pool.tile([P, FD], F32, tag="ps", name="ps_qi")
                    for d in range(DT):
                        nc.tensor.matmul(
                            ps_i[:], wqi_sb[:, d, ts(et, P)], xti_c[:, d, :],
                            start=(d == 0), stop=(d == DT - 1),
                        )
                    nc.vector.tensor_mul(
                        out=qt_sb[:, et, ts(nch, FD)], in0=prt[:], in1=ps_i[:],
                    )

            # ---- kv = Wk^T S Wv from the reduced S ----
            s_sb = sm_pool.tile([P, DT, D], BF16, tag="sm", name="s_sb")
            nc.scalar.dma_start(s_sb[:], as_tiles(bounce_out))

            ut_sb = sm_pool.tile([P, DT, D], BF16, tag="sm", name="ut_sb")
            for dpt in range(DT):      # d' tile (UT partition dim)
                ps_u = [ps_pool.tile([P, FD], F32, tag="ps", name="ps_u")
                        for _ in range(2)]
                for d in range(DT):
                    lhsT = s_sb[:, d, ts(dpt, P)]
                    for kh in range(2):
                        nc.tensor.matmul(
                            ps_u[kh][:], lhsT, wk_sb[:, d, ts(kh, FD)],
                            start=(d == 0), stop=(d == DT - 1),
                        )
                for kh in range(2):
                    nc.vector.tensor_copy(ut_sb[:, dpt, ts(kh, FD)],
                                          ps_u[kh][:])

            kv_sb = sm_pool.tile([P, DT, E], BF16, tag="sm", name="kv_sb")
            for dkt in range(DT):      # dk tile (kv partition dim)
                ps_k = [ps_pool.tile([P, FD], F32, tag="ps", name="ps_k")
                        for _ in range(2)]
                for dp in range(DT):
                    lhsT = ut_sb[:, dp, ts(dkt, P)]
                    for eh in range(2):
                        nc.tensor.matmul(
                            ps_k[eh][:], lhsT, wv_sb[:, dp, ts(eh, FD)],
                            start=(dp == 0), stop=(dp == DT - 1),
                        )
                for eh in range(2):
                    nc.vector.tensor_copy(kv_sb[:, dkt, ts(eh, FD)],
                                          ps_k[eh][:])

            # ---- out = queryT.T @ kv ----
            for nt in range(NT):
                ps_o = [ps_pool.tile([P, FD], F32, tag="ps", name="ps_o")
                        for _ in range(2)]
                for et in range(ET):
                    lhsT = qt_sb[:, et, ts(nt, P)]
                    for eh in range(2):
                        nc.tensor.matmul(
                            ps_o[eh][:], lhsT, kv_sb[:, et, ts(eh, FD)],
                            start=(et == 0), stop=(et == ET - 1),
                        )
                for eh in range(2):
                    o_st = out_pool.tile([P, FD], F32, tag="ost", name="o_st")
                    nc.vector.tensor_copy(o_st[:], ps_o[eh][:])
                    nc.sync.dma_start(out[ts(nt, P), ts(eh, FD)], o_st[:])

    nc.compile()
    return nc


def make_in_maps(x_real, x_imag, w_query_real, w_query_imag, w_key, w_value):
    ws = {
        "wqr": np.ascontiguousarray(w_query_real, dtype=np.float32),
        "wqi": np.ascontiguousarray(w_query_imag, dtype=np.float32),
        "wk": np.ascontiguousarray(w_key, dtype=np.float32),
        "wv": np.ascontiguousarray(w_value, dtype=np.float32),
    }
    in_maps = []
    for c in range(N_CORES):
        b, h = divmod(c, 2)
        sl = slice(h * NL, (h + 1) * NL)
        in_maps.append({
            "xr": np.ascontiguousarray(x_real[b, sl], dtype=np.float32),
            "xi": np.ascontiguousarray(x_imag[b, sl], dtype=np.float32),
            **ws,
        })
    return in_maps


def gather_out(results):
    out = np.empty((B, N, E), np.float32)
    for c in range(N_CORES):
        b, h = divmod(c, 2)
        out[b, h * NL:(h + 1) * NL] = results[c]["out"]
    return out


def kernel(x_real, x_imag, w_query_real, w_query_imag, w_key, w_value):
    nc = build_bass()
    in_maps = make_in_maps(x_real, x_imag, w_query_real, w_query_imag,
                           w_key, w_value)
    res = run_bass_kernel_spmd(nc, in_maps, core_ids=list(range(N_CORES)))
    return gather_out(res.results)


if __name__ == "__main__":
    rng = np.random.default_rng(0)
    args = dict(
        x_real=rng.standard_normal((B, N, D), dtype=np.float32),
        x_imag=rng.standard_normal((B, N, D), dtype=np.float32),
        w_query_real=(rng.standard_normal((D, E), dtype=np.float32) / D),
        w_query_imag=(rng.standard_normal((D, E), dtype=np.float32) / D),
        w_key=(rng.standard_normal((D, E), dtype=np.float32) / D),
        w_value=(rng.standard_normal((D, E), dtype=np.float32) / D),
    )
    got = kernel(**args)
    q = np.einsum("bnd,de->bne", args["x_real"], args["w_query_real"]) * \
        np.einsum("bnd,de->bne", args["x_imag"], args["w_query_imag"])
    k = np.einsum("bnd,de->bne", args["x_real"], args["w_key"])
    v = np.einsum("bnd,de->bne", args["x_imag"], args["w_value"])
    kv = np.einsum("bnd,bne->bde", k, v)
    want = np.einsum("bnd,bde->bne", q, kv)
    denom = np.abs(want).max()
    print("max abs err:", np.abs(got - want).max())
    print("rel err:", np.abs(got - want).max() / denom)


# revision 11
# speedup vs baseline: 1.3909x; 1.0678x over previous
"""BFFN (linear-attention style gated FFN) Trainium2 Bass kernel, 8 NeuronCores.

Reference computation (all fp32, B=4, N=4096, D=E=1024):
    query = (x_real @ Wqr) * (x_imag @ Wqi)        # [b, n, e]
    key   = x_real @ Wk                             # [b, n, d]
    value = x_imag @ Wv                             # [b, n, e]
    kv    = einsum('bnd,bne->bde', key, value)      # [b, d, e]
    out   = einsum('bnd,bde->bne', query, kv)       # [b, n, e]

Key algebraic restructure: kv = Wk^T @ (xr^T @ xi) @ Wv.  With
S = xr^T @ xi (the only sequence-length reduction), the kv path costs
N*D*D + 2*D*D*E instead of 3*N*D*E FLOPs, and S is computed from x in
NATURAL layout (lhsT = xr tile, rhs = xi tile — no transposes needed).

Sharding: 8 cores = 4 batches x 2 sequence-halves. Each pair AllReduces its
partial S (bf16, 2MB) while the query matmuls run; both cores then compute
kv = Wk^T S Wv redundantly (small) and their own half of the output.

S is [1024,1024] fp32 = 4MB > PSUM (2MB), so it accumulates over the
sequence in two passes of 8 PSUM banks each (d' columns 0:512 then
512:1024).  Pass A streams x f32 (HWDGE) with DVE casts to bf16 and writes
a bf16 copy of x back to HBM; pass B re-reads it with no cast.

The query path needs xT [d, n].  xbar DMA transposes turned out to
serialize against ALL concurrent DMA traffic (Tile's deadlock-avoidance
for the xbar), so xT is built on the TENSOR engine (transpose-via-identity)
chunk by chunk inside the query phase, where PSUM banks are free and the
only DMA is the plain bf16 cache re-read.  xT is never fully resident:
each 512-column chunk is transposed, consumed by the query matmuls, and
dropped.
"""
import numpy as np

import concourse.bass as bass
import concourse.mybir as mybir
import concourse.tile as tile
from concourse import bacc
from concourse.bass import ts, ds
from concourse.bass_utils import run_bass_kernel_spmd
from concourse.masks import make_identity

F32 = mybir.dt.float32
BF16 = mybir.dt.bfloat16

B, N, D, E = 4, 4096, 1024, 1024
N_CORES = 8
NL = N // 2          # 2048 rows (sequence) per core
P = 128
NT = NL // P         # 16 n-tiles
DT = D // P          # 8 d tiles
ET = E // P          # 8 e tiles
FD = 512             # matmul moving free dim / PSUM bank
NCH = NL // FD       # 4 n-chunks of 512

REPLICA_GROUPS = [[0, 1], [2, 3], [4, 5], [6, 7]]


def build_bass():
    nc = bacc.Bacc("TRN2", target_bir_lowering=False, debug=False,
                   num_devices=N_CORES)

    xr = nc.dram_tensor("xr", [NL, D], F32, kind="ExternalInput").ap()
    xi = nc.dram_tensor("xi", [NL, D], F32, kind="ExternalInput").ap()
    wqr = nc.dram_tensor("wqr", [D, E], F32, kind="ExternalInput").ap()
    wqi = nc.dram_tensor("wqi", [D, E], F32, kind="ExternalInput").ap()
    wk = nc.dram_tensor("wk", [D, E], F32, kind="ExternalInput").ap()
    wv = nc.dram_tensor("wv", [D, E], F32, kind="ExternalInput").ap()
    out = nc.dram_tensor("out", [NL, E], F32, kind="ExternalOutput").ap()

    def as_tiles(w):  # [1024, n] DRAM view -> [128, 8, n] partition-major
        return w.rearrange("(t p) n -> p t n", p=P)

    with tile.TileContext(nc) as tc:
        with (
            tc.tile_pool(name="xs", bufs=4) as xs_pool,        # f32 staging
            tc.tile_pool(name="xnat", bufs=9) as xnat_pool,    # bf16 x tiles
            tc.tile_pool(name="xnh", bufs=3) as xnh_pool,
            tc.tile_pool(name="xtc", bufs=4) as xtc_pool,      # xT chunks
            tc.tile_pool(name="wp", bufs=4) as w_pool,
            tc.tile_pool(name="qt", bufs=1) as qt_pool,
            tc.tile_pool(name="sm", bufs=2) as sm_pool,
            tc.tile_pool(name="sst", bufs=2) as sst_pool,
            tc.tile_pool(name="prst", bufs=2) as prt_pool,
            tc.tile_pool(name="outst", bufs=2) as out_pool,
            tc.tile_pool(name="cst", bufs=1) as cst_pool,
            tc.tile_pool(name="ps", bufs=8, space="PSUM") as ps_pool,
            tc.tile_pool(name="dram", bufs=2, space="DRAM") as dram_pool,
        ):
            bounce_in = dram_pool.tile([D, D], BF16, name="bounce_in")
            bounce_out = dram_pool.tile([D, D], BF16, name="bounce_out")
            xr_bf = dram_pool.tile([NL, D], BF16, name="xr_bf")
            xi_bf = dram_pool.tile([NL, D], BF16, name="xi_bf")

            ident = cst_pool.tile([P, P], BF16, tag="id", name="ident")
            make_identity(nc, ident)

            def load_w(dram_w, name):  # SWDGE cast-DMA, f32 -> bf16 in flight
                w_sb = w_pool.tile([P, DT, E], BF16, tag="w", name=name)
                nc.gpsimd.dma_start(w_sb[:], as_tiles(dram_w))
                return w_sb

            # ---- pass A over n: load+cast x, S[:, 0:512], write bf16 cache
            ps_s = [ps_pool.tile([P, FD], F32, tag="ps", name="ps_s")
                    for _ in range(DT)]
            w_sbs = {}
            for nt in range(NT):
                xs_r = xs_pool.tile([P, D], F32, tag="xs", name="xs_r")
                nc.scalar.dma_start(xs_r[:], xr[ts(nt, P), :])
                xr_nat = xnat_pool.tile([P, D], BF16, tag="xn", name="xr_nat")
                nc.vector.tensor_copy(xr_nat[:], xs_r[:])
                xs_i = xs_pool.tile([P, D], F32, tag="xs", name="xs_i")
                nc.scalar.dma_start(xs_i[:], xi[ts(nt, P), :])
                xi_nat = xnat_pool.tile([P, D], BF16, tag="xn", name="xi_nat")
                nc.vector.tensor_copy(xi_nat[:], xs_i[:])

                nc.sync.dma_start(xr_bf[ts(nt, P), :], xr_nat[:])
                nc.sync.dma_start(xi_bf[ts(nt, P), :], xi_nat[:])
                for d in range(DT):
                    nc.tensor.matmul(
                        ps_s[d][:], xr_nat[:, ts(d, P)], xi_nat[:, :FD],
                        start=(nt == 0), stop=(nt == NT - 1),
                    )
                if nt % 4 == 1:  # spread the four weight loads across pass A
                    w_name = ("wqr", "wqi", "wk", "wv")[nt // 4]
                    w_sbs[w_name] = load_w({"wqr": wqr, "wqi": wqi,
                                            "wk": wk, "wv": wv}[w_name],
                                           w_name + "_sb")
            for d in range(DT):
                s_st = sst_pool.tile([P, FD], BF16, tag="sst", name="s_st")
                nc.vector.tensor_copy(s_st[:], ps_s[d][:])
                nc.scalar.dma_start(bounce_in[ts(d, P), :FD], s_st[:])

            # ---- pass B over n (bf16 cache reads): S[:, 512:1024] ----
            ps_s2 = [ps_pool.tile([P, FD], F32, tag="ps", name="ps_s2")
                     for _ in range(DT)]
            for nt in range(NT):
                xr_nat2 = xnat_pool.tile([P, D], BF16, tag="xn", name="xr_nat2")
                nc.scalar.dma_start(xr_nat2[:], xr_bf[ts(nt, P), :])
                xi_h = xnh_pool.tile([P, FD], BF16, tag="xnh", name="xi_h")
                nc.scalar.dma_start(xi_h[:], xi_bf[ts(nt, P), FD:])
                for d in range(DT):
                    nc.tensor.matmul(
                        ps_s2[d][:], xr_nat2[:, ts(d, P)], xi_h[:],
                        start=(nt == 0), stop=(nt == NT - 1),
                    )
            for d in range(DT):
                s_st2 = sst_pool.tile([P, FD], BF16, tag="sst", name="s_st2")
                nc.vector.tensor_copy(s_st2[:], ps_s2[d][:])
                nc.scalar.dma_start(bounce_in[ts(d, P), FD:], s_st2[:])

            # ---- pairwise AllReduce of S (bf16, 2MB); overlaps query ----
            nc.gpsimd.collective_compute(
                "AllReduce",
                mybir.AluOpType.add,
                replica_groups=REPLICA_GROUPS,
                ins=[bounce_in.opt()],
                outs=[bounce_out.opt()],
            )

            # ---- query phase, chunked: PE-transpose xT chunk, then MMs ----
            wqr_sb, wqi_sb = w_sbs["wqr"], w_sbs["wqi"]
            wk_sb, wv_sb = w_sbs["wk"], w_sbs["wv"]
            qt_sb = qt_pool.tile([P, ET, NL], BF16, tag="qt", name="qt_sb")
            for nch in range(NCH):
                chunk_xt = {}
                for src_bf, kind in ((xr_bf, "r"), (xi_bf, "i")):
                    nats = []
                    for j in range(4):  # re-read 4 bf16 x tiles of this chunk
                        xn = xnat_pool.tile([P, D], BF16, tag="xn",
                                            name="xn_c")
                        nc.scalar.dma_start(
                            xn[:], src_bf[ts(4 * nch + j, P), :])
                        nats.append(xn)
                    xt_c = xtc_pool.tile([P, DT, FD], BF16, tag="xtc",
                                         name="xt_c")
                    for d in range(DT):
                        ps_t = ps_pool.tile([P, FD], BF16, tag="ps",
                                            name="ps_t")
                        for j in range(4):
                            nc.tensor.transpose(
                                ps_t[:, ts(j, P)], nats[j][:, ts(d, P)],
                                ident[:],
                            )
                        nc.vector.tensor_copy(xt_c[:, d, :], ps_t[:])
                    chunk_xt[kind] = xt_c
                xtr_c, xti_c = chunk_xt["r"], chunk_xt["i"]
                for et in range(ET):
                    ps_r = ps_pool.tile([P, FD], F32, tag="ps", name="ps_qr")
                    for d in range(DT):
                        nc.tensor.matmul(
                            ps_r[:], wqr_sb[:, d, ts(et, P)], xtr_c[:, d, :],
                            start=(d == 0), stop=(d == DT - 1),
                        )
                    prt = prt_pool.tile([P, FD], BF16, tag="prt", name="prt")
                    nc.vector.tensor_copy(prt[:], ps_r[:])
                    ps_i = ps_pool.tile([P, FD], F32, tag="ps", name="ps_qi")
                    for d in range(DT):
                        nc.tensor.matmul(
                            ps_i[:], wqi_sb[:, d, ts(et, P)], xti_c[:, d, :],
                            start=(d == 0), stop=(d == DT - 1),
                        )
                    nc.vector.tensor_mul(
                        out=qt_sb[:, et, ts(nch, FD)], in0=prt[:], in1=ps_i[:],
                    )

            # ---- kv = Wk^T S Wv from the reduced S ----
            s_sb = sm_pool.tile([P, DT, D], BF16, tag="sm", name="s_sb")
            nc.scalar.dma_start(s_sb[:], as_tiles(bounce_out))

            ut_sb = sm_pool.tile([P, DT, D], BF16, tag="sm", name="ut_sb")
            for dpt in range(DT):      # d' tile (UT partition dim)
                ps_u = [ps_pool.tile([P, FD], F32, tag="ps", name="ps_u")
                        for _ in range(2)]
                for d in range(DT):
                    lhsT = s_sb[:, d, ts(dpt, P)]
                    for kh in range(2):
                        nc.tensor.matmul(
                            ps_u[kh][:], lhsT, wk_sb[:, d, ts(kh, FD)],
                            start=(d == 0), stop=(d == DT - 1),
                        )
                for kh in range(2):
                    nc.vector.tensor_copy(ut_sb[:, dpt, ts(kh, FD)],
                                          ps_u[kh][:])

            kv_sb = sm_pool.tile([P, DT, E], BF16, tag="sm", name="kv_sb")
            for dkt in range(DT):      # dk tile (kv partition dim)
                ps_k = [ps_pool.tile([P, FD], F32, tag="ps", name="ps_k")
                        for _ in range(2)]
                for dp in range(DT):
                    lhsT = ut_sb[:, dp, ts(dkt, P)]
                    for eh in range(2):
                        nc.tensor.matmul(
                            ps_k[eh][:], lhsT, wv_sb[:, dp, ts(eh, FD)],
                            start=(dp == 0), stop=(dp == DT - 1),
                        )
                for eh in range(2):
                    nc.vector.tensor_copy(kv_sb[:, dkt, ts(eh, FD)],
                                          ps_k[eh][:])

            # ---- out = queryT.T @ kv ----
            for nt in range(NT):
                ps_o = [ps_pool.tile([P, FD], F32, tag="ps", name="ps_o")
                        for _ in range(2)]
                for et in range(ET):
                    lhsT = qt_sb[:, et, ts(nt, P)]
                    for eh in range(2):
                        nc.tensor.matmul(
                            ps_o[eh][:], lhsT, kv_sb[:, et, ts(eh, FD)],
                            start=(et == 0), stop=(et == ET - 1),
                        )
                for eh in range(2):
                    o_st = out_pool.tile([P, FD], F32, tag="ost", name="o_st")
                    nc.vector.tensor_copy(o_st[:], ps_o[eh][:])
                    nc.sync.dma_start(out[ts(nt, P), ts(eh, FD)], o_st[:])

    nc.compile()
    return nc


def make_in_maps(x_real, x_imag, w_query_real, w_query_imag, w_key, w_value):
    ws = {
        "wqr": np.ascontiguousarray(w_query_real, dtype=np.float32),
        "wqi": np.ascontiguousarray(w_query_imag, dtype=np.float32),
        "wk": np.ascontiguousarray(w_key, dtype=np.float32),
        "wv": np.ascontiguousarray(w_value, dtype=np.float32),
    }
    in_maps = []
    for c in range(N_CORES):
        b, h = divmod(c, 2)
        sl = slice(h * NL, (h + 1) * NL)
        in_maps.append({
            "xr": np.ascontiguousarray(x_real[b, sl], dtype=np.float32),
            "xi": np.ascontiguousarray(x_imag[b, sl], dtype=np.float32),
            **ws,
        })
    return in_maps


def gather_out(results):
    out = np.empty((B, N, E), np.float32)
    for c in range(N_CORES):
        b, h = divmod(c, 2)
        out[b, h * NL:(h + 1) * NL] = results[c]["out"]
    return out


def kernel(x_real, x_imag, w_query_real, w_query_imag, w_key, w_value):
    nc = build_bass()
    in_maps = make_in_maps(x_real, x_imag, w_query_real, w_query_imag,
                           w_key, w_value)
    res = run_bass_kernel_spmd(nc, in_maps, core_ids=list(range(N_CORES)))
    return gather_out(res.results)


if __name__ == "__main__":
    rng = np.random.default_rng(0)
    args = dict(
        x_real=rng.standard_normal((B, N, D), dtype=np.float32),
        x_imag=rng.standard_normal((B, N, D), dtype=np.float32),
        w_query_real=(rng.standard_normal((D, E), dtype=np.float32) / D),
        w_query_imag=(rng.standard_normal((D, E), dtype=np.float32) / D),
        w_key=(rng.standard_normal((D, E), dtype=np.float32) / D),
        w_value=(rng.standard_normal((D, E), dtype=np.float32) / D),
    )
    got = kernel(**args)
    q = np.einsum("bnd,de->bne", args["x_real"], args["w_query_real"]) * \
        np.einsum("bnd,de->bne", args["x_imag"], args["w_query_imag"])
    k = np.einsum("bnd,de->bne", args["x_real"], args["w_key"])
    v = np.einsum("bnd,de->bne", args["x_imag"], args["w_value"])
    kv = np.einsum("bnd,bne->bde", k, v)
    want = np.einsum("bnd,bde->bne", q, kv)
    denom = np.abs(want).max()
    print("max abs err:", np.abs(got - want).max())
    print("rel err:", np.abs(got - want).max() / denom)
